# revision 1
# baseline (speedup 1.0000x reference)
"""Trainium2 Bass kernel for nn_ADCLayer (GAT-style message passing).

Math (reference reduction):
  sj = X @ (Wv @ aw[:d]) + bv.aw[:d]          (per-column score, j axis)
  si = X @ (Wv @ aw[d:]) + bv.aw[d:] + ab     (per-row score, i axis)
  alpha = A * exp(leaky_relu(si[i] + sj[j]))  (unnormalized transition)
  T = alpha / rowsum(alpha)
  H = X@Wk0 + (T X)@Wk1 + (T^2 X)@Wk2 + sum_k bk[k]   (last ref hop is dead code)
  out = relu(H)

Device algebra (per core, partition=j layout, zero big transposes, both
hops run on RAW alphaT so nothing waits for normalization):
  alphaT[j, i] = A^T[j, i] * exp(lrelu(si[i] + sj[j]))      (bf16)
  r via ones-stationary matmul -> (1, I); recip_col via 8 tiny PE
  transposes + exact reciprocal; r128 row-broadcast via ones outer-product.
  P2 = X@Wk2 ; G2 = recip_col x (alphaT^T P2) + bks -> pairwise AllGather.
  P1 = X@Wk1 ; S-own = P1 + G2-own ; S-other = P1 + masked partner half.
  H_psum = (r x X)@Wk0 + alphaT^T S ; out = relu(recip_col * H_psum).

Sharding: 8 cores = 4 batches x 2 row-halves; j axis permuted per core
(own half first) so own j-tiles have uniform local indices.

DMA strategy: few big multi-dim DMAs (issue op costs ~0.65us of engine
time each), priority emission order, all on the sync queue.
"""

import numpy as np

B, N, DIN, DOUT = 4, 2048, 512, 512
HALF = N // 2          # rows per core
NCORES = 8
JT = N // 128          # 16 j tiles
IT = HALF // 128       # 8 i tiles (also own j tiles)
DT = DIN // 128        # 4 d tiles

_CACHE = {}


def _build():
    import concourse.bacc as bacc
    import concourse.tile as tile
    import concourse.mybir as mybir
    from concourse.bass import ds, ts
    from concourse.tile_rust import add_dep_helper

    f32 = mybir.dt.float32
    bf16 = mybir.dt.bfloat16
    AOP = mybir.AluOpType
    AF = mybir.ActivationFunctionType

    nc = bacc.Bacc("TRN2", target_bir_lowering=False, debug=False,
                   num_devices=NCORES)

    AT = nc.declare_dram_parameter("AT", [N, HALF], bf16, isOutput=False)
    XT = nc.declare_dram_parameter("XT", [DIN, N], bf16, isOutput=False)
    WK0 = nc.declare_dram_parameter("WK0", [DIN, DOUT], bf16, isOutput=False)
    WK1 = nc.declare_dram_parameter("WK1", [DIN, DOUT], bf16, isOutput=False)
    WK2 = nc.declare_dram_parameter("WK2", [DIN, DOUT], bf16, isOutput=False)
    SJT = nc.declare_dram_parameter("SJT", [128, JT], f32, isOutput=False)
    SIB = nc.declare_dram_parameter("SIB", [128, HALF], f32, isOutput=False)
    BKS = nc.declare_dram_parameter("BKS", [128, DOUT], f32, isOutput=False)
    MLO = nc.declare_dram_parameter("MLO", [1, 1], f32, isOutput=False)
    MHI = nc.declare_dram_parameter("MHI", [1, 1], f32, isOutput=False)
    OUT = nc.declare_dram_parameter("out", [HALF, DOUT], f32, isOutput=True)

    g2_in = nc.dram_tensor("g2_in", [HALF, DOUT], bf16)
    g2_all = nc.dram_tensor("g2_all", [N, DOUT], bf16)

    with tile.TileContext(nc) as tc:
        with tc.tile_pool(name="sb", bufs=1) as sb:
            # ---- input DMAs: fast class unchained, rest chained --------
            fast = []
            sjt = sb.tile([128, JT], f32, tag="sjt", bufs=1)
            fast.append(nc.sync.dma_start(out=sjt[:], in_=SJT[:, :]))
            sib = sb.tile([128, HALF], f32, tag="sib", bufs=1)
            fast.append(nc.sync.dma_start(out=sib[:], in_=SIB[:, :]))
            at = []
            for jt in range(2):
                t = sb.tile([128, HALF], bf16, tag=f"at{jt}", bufs=1)
                fast.append(nc.sync.dma_start(out=t[:], in_=AT[ts(jt, 128), :]))
                at.append(t)
            xt = []
            for d in range(DT):
                t = sb.tile([128, N], bf16, tag=f"xt{d}", bufs=1)
                fast.append(nc.sync.dma_start(out=t[:], in_=XT[ts(d, 128), :]))
                xt.append(t)
            wk = {}
            for nm, src_ in (("wk2", WK2),):
                wk[nm] = []
                for d in range(DT):
                    t = sb.tile([128, DOUT], bf16, tag=f"{nm}_{d}", bufs=1)
                    fast.append(nc.sync.dma_start(out=t[:], in_=src_[ts(d, 128), :]))
                    wk[nm].append(t)

            def chain(dma):
                for p in fast:
                    add_dep_helper(dma.ins, p.ins, reason="dma priority")
                return dma

            for nm, src_ in (("wk1", WK1), ("wk0", WK0)):
                wk[nm] = []
                for d in range(DT):
                    t = sb.tile([128, DOUT], bf16, tag=f"{nm}_{d}", bufs=1)
                    chain(nc.sync.dma_start(out=t[:], in_=src_[ts(d, 128), :]))
                    wk[nm].append(t)
            for jt in range(2, JT):
                t = sb.tile([128, HALF], bf16, tag=f"at{jt}", bufs=1)
                chain(nc.sync.dma_start(out=t[:], in_=AT[ts(jt, 128), :]))
                at.append(t)
            bks = sb.tile([128, DOUT], f32, tag="bks", bufs=1)
            chain(nc.sync.dma_start(out=bks[:], in_=BKS[:, :]))
            mlo = sb.tile([128, 1], f32, tag="mlo", bufs=1)
            chain(nc.sync.dma_start(out=mlo[:],
                                    in_=MLO[:, :].to_broadcast((128, 1))))
            mhi = sb.tile([128, 1], f32, tag="mhi", bufs=1)
            chain(nc.sync.dma_start(out=mhi[:],
                                    in_=MHI[:, :].to_broadcast((128, 1))))
            ones = sb.tile([128, 1], bf16, tag="ones", bufs=1)
            nc.vector.memset(ones[:], 1.0)
            ones1f = sb.tile([1, 1], f32, tag="ones1f", bufs=1)
            nc.vector.memset(ones1f[:], 1.0)
            ones128f = sb.tile([1, 128], f32, tag="ones128f", bufs=1)
            nc.vector.memset(ones128f[:], 1.0)

            def atS(jt, off, size):
                return at[jt][:, off: off + size]

            p1 = [None] * JT
            with tc.tile_pool(name="psAll", bufs=1, space="PSUM") as psA:
                # ---- phase 1: elementwise alphaT (raw) + r + P2 --------
                r_ps = psA.tile([1, HALF], f32, tag="r", bufs=1)
                for jt in range(JT):
                    z = sb.tile([128, HALF], f32, tag="z", bufs=3)
                    nc.scalar.activation(z[:], sib[:], AF.Identity,
                                         bias=sjt[:, jt:jt + 1], scale=1.0)
                    nc.vector.scalar_tensor_tensor(z[:], z[:], 0.2, z[:],
                                                   op0=AOP.mult, op1=AOP.max)
                    e = sb.tile([128, HALF], bf16, tag="e", bufs=3)
                    nc.scalar.activation(e[:], z[:], AF.Exp)
                    nc.vector.tensor_mul(atS(jt, 0, HALF), atS(jt, 0, HALF),
                                         e[:])
                    for h in range(2):
                        nc.tensor.matmul(r_ps[:, ts(h, 512)], lhsT=ones[:],
                                         rhs=atS(jt, h * 512, 512),
                                         start=(jt == 0), stop=(jt == JT - 1))

                p2 = []
                for jt in range(JT):
                    pp2 = psA.tile([128, DOUT], f32, tag="mm", bufs=2)
                    for d in range(DT):
                        nc.tensor.matmul(pp2[:], lhsT=xt[d][:, ts(jt, 128)],
                                         rhs=wk["wk2"][d][:],
                                         start=(d == 0), stop=(d == DT - 1))
                    t2 = sb.tile([128, DOUT], bf16, tag=f"p2_{jt}", bufs=1)
                    nc.vector.tensor_copy(t2[:], pp2[:])
                    p2.append(t2)

                # ---- hop A on RAW alphaT: two 4-bank passes ------------
                g2sb = [None] * IT
                ua = [None] * IT
                for half in range(2):
                    for i in range(4):
                        it = half * 4 + i
                        ua[it] = psA.tile([128, DOUT], f32, tag=f"ua{i}",
                                          bufs=1, name=f"ua{half}_{i}")
                    for jt in range(JT):
                        for i in range(4):
                            it = half * 4 + i
                            nc.tensor.matmul(ua[it][:],
                                             lhsT=atS(jt, it * 128, 128),
                                             rhs=p2[jt][:],
                                             start=(jt == 0),
                                             stop=(jt == JT - 1))

                    if half == 0:
                        # r -> recip_col (8 tiny PE transposes + recip)
                        r_sb = sb.tile([1, HALF], f32, tag="rsb", bufs=1)
                        nc.vector.tensor_copy(r_sb[:], r_ps[:])
                        rt = psA.tile([128, IT], f32, tag="r", bufs=1,
                                      name="rt")
                        for c in range(IT):
                            nc.tensor.matmul(rt[:, c:c + 1],
                                             lhsT=r_sb[0:1, ts(c, 128)],
                                             rhs=ones1f[:],
                                             is_transpose=True,
                                             start=True, stop=True)
                        rr_col = sb.tile([128, IT], f32, tag="rrc", bufs=1)
                        nc.vector.reciprocal(rr_col[:], rt[:])

                    # G2 = recip_col * UA + bks, straight to the gather
                    for i in range(4):
                        it = half * 4 + i
                        g2t = sb.tile([128, DOUT], bf16, tag=f"g2o{it}",
                                      bufs=1, name=f"g2o{it}")
                        nc.vector.scalar_tensor_tensor(
                            g2t[:], ua[it][:], rr_col[:, it:it + 1], bks[:],
                            op0=AOP.mult, op1=AOP.add)
                        g2sb[it] = g2t
                        eng = nc.scalar if it % 2 else nc.sync
                        eng.dma_start(out=g2_in[ts(it, 128), :],
                                      in_=g2t[:])

                nc.gpsimd.collective_compute(
                    "AllGather", AOP.bypass,
                    ins=[g2_in.ap().opt()],
                    outs=[g2_all.ap().opt()],
                    replica_groups=[[0, 1], [2, 3], [4, 5], [6, 7]],
                )

                # r128 row-broadcast via ones outer product (into psum)
                r128 = psA.tile([128, HALF], f32, tag="r", bufs=1,
                                name="r128")
                for h in range(2):
                    nc.tensor.matmul(r128[:, ts(h, 512)], lhsT=ones128f[:],
                                     rhs=r_sb[0:1, ts(h, 512)],
                                     start=True, stop=True)
                xts = []
                for d in range(DT):
                    t = sb.tile([128, HALF], bf16, tag=f"xts{d}", bufs=1)
                    nc.vector.tensor_mul(t[:], xt[d][:, 0:HALF], r128[:])
                    xts.append(t)

                # ---- cover window: P1 (+S-own fused) -------------------
                for jt in range(JT):
                    pp1 = psA.tile([128, DOUT], f32, tag="mm", bufs=2)
                    for d in range(DT):
                        nc.tensor.matmul(pp1[:], lhsT=xt[d][:, ts(jt, 128)],
                                         rhs=wk["wk1"][d][:],
                                         start=(d == 0), stop=(d == DT - 1))
                    t1 = sb.tile([128, DOUT], bf16, tag=f"p1_{jt}", bufs=1)
                    if jt < IT:
                        nc.vector.scalar_tensor_tensor(
                            t1[:], pp1[:], 1.0, g2sb[jt][:],
                            op0=AOP.mult, op1=AOP.add)
                    else:
                        nc.vector.tensor_copy(t1[:], pp1[:])
                    p1[jt] = t1

            # ---- S-other: partner half via masked add -----------------
            for k in range(IT):
                glo = sb.tile([128, DOUT], bf16, tag=f"glo{k}", bufs=1,
                              name=f"glo{k}")
                nc.scalar.dma_start(out=glo[:], in_=g2_all[ts(k, 128), :])
                ghi = sb.tile([128, DOUT], bf16, tag=f"ghi{k}", bufs=1,
                              name=f"ghi{k}")
                nc.sync.dma_start(out=ghi[:], in_=g2_all[ts(IT + k, 128), :])
                stmp = sb.tile([128, DOUT], bf16, tag="stmp", bufs=3)
                nc.vector.scalar_tensor_tensor(stmp[:], glo[:], mlo[:, 0:1],
                                               p1[IT + k][:],
                                               op0=AOP.mult, op1=AOP.add)
                nc.vector.scalar_tensor_tensor(p1[IT + k][:], ghi[:],
                                               mhi[:, 0:1], stmp[:],
                                               op0=AOP.mult, op1=AOP.add)

            # ---- phase 3: H = (r x X)@Wk0 + alphaT^T S ----------------
            with tc.tile_pool(name="psC", bufs=1, space="PSUM") as psC:
                hps = [psC.tile([128, DOUT], f32, tag=f"h{i}", bufs=1,
                               name=f"h{i}") for i in range(IT)]
                for it in range(IT):
                    for d in range(DT):
                        nc.tensor.matmul(hps[it][:],
                                         lhsT=xts[d][:, ts(it, 128)],
                                         rhs=wk["wk0"][d][:],
                                         start=(d == 0), stop=False)
                for jt in range(IT):
                    for it in range(IT):
                        nc.tensor.matmul(hps[it][:],
                                         lhsT=atS(jt, it * 128, 128),
                                         rhs=p1[jt][:],
                                         start=False, stop=False)
                for it_half in (range(0, IT // 2), range(IT // 2, IT)):
                    for it in it_half:
                        for jt in range(IT, JT):
                            nc.tensor.matmul(hps[it][:],
                                             lhsT=atS(jt, it * 128, 128),
                                             rhs=p1[jt][:],
                                             start=False,
                                             stop=(jt == JT - 1))
                        o = sb.tile([128, DOUT], f32, tag="osb", bufs=3)
                        nc.scalar.activation(o[:], hps[it][:], AF.Relu,
                                             scale=rr_col[:, it:it + 1])
                        nc.sync.dma_start(out=OUT[ts(it, 128), :], in_=o[:])

    nc.compile()
    return nc


def _prep_inputs(X, A, Wv, bv, aw, ab, Wk, bk):
    import ml_dtypes

    bf16 = ml_dtypes.bfloat16
    X = np.asarray(X, np.float32)
    A = np.asarray(A, np.float32)
    Wv = np.asarray(Wv, np.float32)
    bv = np.asarray(bv, np.float32)
    aw = np.asarray(aw, np.float32)
    ab = np.asarray(ab, np.float32)
    Wk = np.asarray(Wk, np.float32)
    bk = np.asarray(bk, np.float32)

    w1 = Wv @ aw[:DOUT, 0]
    c1 = float(bv @ aw[:DOUT, 0])
    w2 = Wv @ aw[DOUT:, 0]
    c2 = float(bv @ aw[DOUT:, 0]) + float(ab[0])
    bks = bk.sum(axis=0).astype(np.float32)
    bks128 = np.ascontiguousarray(np.broadcast_to(bks[None, :], (128, DOUT)),
                                  dtype=np.float32)

    wk_b = [np.ascontiguousarray(Wk[k]).astype(bf16) for k in range(3)]
    in_maps = []
    for c in range(NCORES):
        b, hf = c // 2, c % 2
        own = slice(hf * HALF, (hf + 1) * HALF)
        oth = slice((1 - hf) * HALF, (2 - hf) * HALF)
        perm = np.r_[np.arange(own.start, own.stop),
                     np.arange(oth.start, oth.stop)]
        Xb = X[b]
        sj = (Xb @ w1 + c1).astype(np.float32)
        si = (Xb @ w2 + c2).astype(np.float32)
        sib128 = np.ascontiguousarray(
            np.broadcast_to(si[own][None, :], (128, HALF)), dtype=np.float32)
        in_maps.append({
            "AT": np.ascontiguousarray(A[b][own, :].T[perm, :]).astype(bf16),
            "XT": np.ascontiguousarray(Xb.T[:, perm]).astype(bf16),
            "WK0": wk_b[0], "WK1": wk_b[1], "WK2": wk_b[2],
            "SJT": np.ascontiguousarray(sj[perm].reshape(JT, 128).T,
                                        np.float32),
            "SIB": sib128,
            "BKS": bks128,
            "MLO": np.full((1, 1), 1.0 if hf == 1 else 0.0, np.float32),
            "MHI": np.full((1, 1), 1.0 if hf == 0 else 0.0, np.float32),
        })
    return in_maps


LAST_RESULTS = None


def kernel(X, A, Wv, bv, aw, ab, Wk, bk):
    from concourse.bass_utils import run_bass_kernel_spmd

    if "nc" not in _CACHE:
        _CACHE["nc"] = _build()
    nc = _CACHE["nc"]

    in_maps = _prep_inputs(X, A, Wv, bv, aw, ab, Wk, bk)
    try:
        res = run_bass_kernel_spmd(nc, in_maps, core_ids=list(range(NCORES)))
    except Exception:
        import time
        time.sleep(20)
        res = run_bass_kernel_spmd(nc, in_maps, core_ids=list(range(NCORES)))
    global LAST_RESULTS
    LAST_RESULTS = res

    out = np.empty((B, N, DOUT), np.float32)
    for c in range(NCORES):
        b, hf = c // 2, c % 2
        out[b, hf * HALF:(hf + 1) * HALF, :] = res.results[c]["out"]
    return out



# revision 6
# speedup vs baseline: 1.0750x; 1.0750x over previous
"""Trainium2 Bass kernel for nn_ADCLayer (GAT-style message passing).

Math (reference reduction):
  sj = X @ (Wv @ aw[:d]) + bv.aw[:d]          (per-column score, j axis)
  si = X @ (Wv @ aw[d:]) + bv.aw[d:] + ab     (per-row score, i axis)
  alpha = A * exp(leaky_relu(si[i] + sj[j]))  (unnormalized transition)
  T = alpha / rowsum(alpha)
  H = X@Wk0 + (T X)@Wk1 + (T^2 X)@Wk2 + sum_k bk[k]   (last ref hop is dead code)
  out = relu(H)

Key identity used on device: exp is monotone, so
  exp(lrelu(x)) = max(exp(x), exp(0.2 x)),  and with x = si + sj both
  branches are rank-1:  exp(si+sj) = exp(si)*exp(sj).
The host precomputes u1=exp(sj), u2=exp(0.2 sj) (per-partition columns)
and V1=exp(si), V2=exp(0.2 si) (broadcast rows), so the device per j-tile
does just: m1 = u1*V1 (scalar engine), m2 = max(u2*V2, m1) (DVE stt),
alphaT = A*m2 (DVE) -- 3 cheap bf16 passes, no Exp LUT.

Device algebra (per core, partition=j layout, zero big transposes, both
hops run on RAW alphaT so nothing waits for normalization):
  alphaT[j, i] = A^T[j, i] * max(u1[j]V1[i], u2[j]V2[i])   (bf16)
  r via ones-stationary matmul -> (1, I); rr_col via 8 tiny PE
  transposes + exact reciprocal.
  P2 = X@Wk2 ; G2 = rr_col * (alphaT^T P2) + bks.
  G2-own gathered pairwise in two halves (pipelined collectives).
  S = P1 + G2 (own fused from PSUM; partner via masked add).
  H_psum = (r*X)@Wk0 + alphaT^T S ; out = relu(rr_col * H_psum), bf16.

Sharding: 8 cores = 4 batches x 2 row-halves; j axis permuted per core
(own half first) so own j-tiles have uniform local indices.

Schedule: batched multi-megabyte DMAs (few issue ops), PE emission order
r/P2 interleaved with the elementwise pipeline, hopA in two 4-bank PSUM
halves each followed by its own half-gather, P1 emitted partner-half
first (plain copies) then own-half (fused S-own adds), hopB j-chunked
(own j, then gather0 tiles, then gather1 tiles) with per-i relu+DMA
dribble at the end.
"""

import numpy as np

B, N, DIN, DOUT = 4, 2048, 512, 512
HALF = N // 2          # rows per core
NCORES = 8
JT = N // 128          # 16 j tiles
IT = HALF // 128       # 8 i tiles (also own j tiles)
DT = DIN // 128        # 4 d tiles

_CACHE = {}


def _build():
    import concourse.bacc as bacc
    import concourse.tile as tile
    import concourse.mybir as mybir
    from concourse.bass import ds, ts

    f32 = mybir.dt.float32
    bf16 = mybir.dt.bfloat16
    AOP = mybir.AluOpType
    AF = mybir.ActivationFunctionType

    nc = bacc.Bacc("TRN2", target_bir_lowering=False, debug=False,
                   num_devices=NCORES)

    # Host-interleaved layouts: [...] dims are (partition, tile, col) flat.
    ATH = nc.declare_dram_parameter("ATH", [128, JT * HALF], bf16,
                                    isOutput=False)
    XTH = nc.declare_dram_parameter("XTH", [128, DT * N], bf16,
                                    isOutput=False)
    # wk order: wk2 d0..3, wk1 d0..3, wk0 d0..3
    WKH = nc.declare_dram_parameter("WKH", [128, 12 * 512], bf16,
                                    isOutput=False)
    # smalls: u1(16) u2(16) bks(512) mlo(1) mhi(1)
    SM = nc.declare_dram_parameter("SM", [128, 546], f32, isOutput=False)
    # V1(1024) V2(1024) broadcast rows
    VV = nc.declare_dram_parameter("VV", [128, 2 * HALF], bf16,
                                   isOutput=False)
    OUT = nc.declare_dram_parameter("out", [HALF, DOUT], bf16, isOutput=True)

    g_in0 = nc.dram_tensor("g_in0", [128, 4 * 512], bf16)
    g_all0 = nc.dram_tensor("g_all0", [256, 4 * 512], bf16)
    g_in1 = nc.dram_tensor("g_in1", [128, 4 * 512], bf16)
    g_all1 = nc.dram_tensor("g_all1", [256, 4 * 512], bf16)

    GROUPS = [[0, 1], [2, 3], [4, 5], [6, 7]]

    with tile.TileContext(nc) as tc:
        with tc.tile_pool(name="sb", bufs=1) as sb:
            # ---- big SBUF tiles ---------------------------------------
            at_all = sb.tile([128, JT * HALF], bf16, tag="at", bufs=1)
            xt_all = sb.tile([128, DT * N], bf16, tag="xt", bufs=1)
            wk_all = sb.tile([128, 12 * 512], bf16, tag="wk", bufs=1)
            sm = sb.tile([128, 546], f32, tag="sm", bufs=1)
            vv = sb.tile([128, 2 * HALF], bf16, tag="vv", bufs=1)
            p2_all = sb.tile([128, JT * 512], bf16, tag="p2", bufs=1)
            s_all = sb.tile([128, JT * 512], bf16, tag="s", bufs=1)
            g2o = sb.tile([128, IT * 512], bf16, tag="g2o", bufs=1)
            glo0 = sb.tile([128, 4 * 512], bf16, tag="glo0", bufs=1)
            ghi0 = sb.tile([128, 4 * 512], bf16, tag="ghi0", bufs=1)
            glo1 = sb.tile([128, 4 * 512], bf16, tag="glo1", bufs=1)
            ghi1 = sb.tile([128, 4 * 512], bf16, tag="ghi1", bufs=1)
            xts_all = sb.tile([128, DT * HALF], bf16, tag="xts", bufs=1)
            o_all = sb.tile([128, IT * 512], bf16, tag="o", bufs=1)
            r_sb = sb.tile([1, HALF], f32, tag="rsb", bufs=1)
            r_sbh = sb.tile([1, HALF], bf16, tag="rsbh", bufs=1)
            rr_col = sb.tile([128, IT], f32, tag="rrc", bufs=1)

            # ---- input DMAs (few big issues, 2 queues) ----------------
            nc.sync.dma_start(out=sm[:], in_=SM[:, :])
            nc.sync.dma_start(out=vv[:], in_=VV[:, :])
            nc.sync.dma_start(out=at_all[:, 0:2 * HALF],
                              in_=ATH[:, 0:2 * HALF])
            nc.sync.dma_start(out=at_all[:, 2 * HALF:8 * HALF],
                              in_=ATH[:, 2 * HALF:8 * HALF])
            nc.sync.dma_start(out=at_all[:, 8 * HALF:JT * HALF],
                              in_=ATH[:, 8 * HALF:JT * HALF])
            nc.scalar.dma_start(out=wk_all[:, 0:4 * 512],
                                in_=WKH[:, 0:4 * 512])
            nc.scalar.dma_start(out=xt_all[:], in_=XTH[:, :])
            nc.scalar.dma_start(out=wk_all[:, 4 * 512:12 * 512],
                                in_=WKH[:, 4 * 512:12 * 512])

            ones = sb.tile([128, 1], bf16, tag="ones", bufs=1)
            nc.vector.memset(ones[:], 1.0)
            ones1f = sb.tile([1, 1], f32, tag="ones1f", bufs=1)
            nc.vector.memset(ones1f[:], 1.0)
            ones128h = sb.tile([1, 128], bf16, tag="o128", bufs=1)
            nc.vector.memset(ones128h[:], 1.0)

            def atS(jt, off, size):
                return at_all[:, jt * HALF + off: jt * HALF + off + size]

            def p2S(jt):
                return p2_all[:, jt * 512:(jt + 1) * 512]

            def sS(jt):
                return s_all[:, jt * 512:(jt + 1) * 512]

            V1 = vv[:, 0:HALF]
            V2 = vv[:, HALF:2 * HALF]
            bks = sm[:, 32:544]
            mlo = sm[:, 544:545]
            mhi = sm[:, 545:546]

            with tc.tile_pool(name="psA", bufs=1, space="PSUM") as psA:
                # ---- phase 1: elementwise alphaT + r + P2 -------------
                r_ps = psA.tile([1, HALF], f32, tag="r", bufs=1)
                pp2_tiles = []
                for jt in range(JT):
                    m1 = sb.tile([128, HALF], bf16, tag="m1", bufs=3)
                    nc.scalar.activation(m1[:], V1, AF.Identity,
                                         scale=sm[:, jt:jt + 1])
                    m2 = sb.tile([128, HALF], bf16, tag="m2", bufs=3)
                    nc.vector.scalar_tensor_tensor(
                        m2[:], V2, sm[:, 16 + jt:17 + jt], m1[:],
                        op0=AOP.mult, op1=AOP.max)
                    nc.vector.tensor_mul(atS(jt, 0, HALF), atS(jt, 0, HALF),
                                         m2[:])
                    # r rowsum (PE, interleaved with P2)
                    for h in range(2):
                        nc.tensor.matmul(r_ps[:, ts(h, 512)], lhsT=ones[:],
                                         rhs=atS(jt, h * 512, 512),
                                         start=(jt == 0), stop=(jt == JT - 1))
                    # P2 tile jt (psum->sbuf copy deferred 2 tiles so the
                    # scalar queue never head-blocks the elementwise pipe)
                    if jt >= 2:
                        nc.scalar.copy(p2S(jt - 2), pp2_tiles[jt - 2][:])
                    pp2 = psA.tile([128, DOUT], f32, tag="mm", bufs=2,
                                   name=f"pp2_{jt}")
                    for d in range(DT):
                        nc.tensor.matmul(
                            pp2[:],
                            lhsT=xt_all[:, d * N + jt * 128:
                                        d * N + (jt + 1) * 128],
                            rhs=wk_all[:, d * 512:(d + 1) * 512],
                            start=(d == 0), stop=(d == DT - 1))
                    pp2_tiles.append(pp2)

                nc.scalar.copy(p2S(JT - 2), pp2_tiles[JT - 2][:])
                nc.scalar.copy(p2S(JT - 1), pp2_tiles[JT - 1][:])

                # ---- hop A half 0 (i-tiles 0-3) -----------------------
                ua0 = [psA.tile([128, DOUT], f32, tag=f"ua{i}", bufs=1,
                                name=f"ua0_{i}") for i in range(4)]
                for jt in range(JT):
                    for i in range(4):
                        nc.tensor.matmul(ua0[i][:],
                                         lhsT=atS(jt, i * 128, 128),
                                         rhs=p2S(jt),
                                         start=(jt == 0),
                                         stop=(jt == JT - 1))

                # r -> rr_col (8 tiny PE transposes + exact reciprocal)
                nc.vector.tensor_copy(r_sb[:], r_ps[:])
                rt = psA.tile([128, IT], f32, tag="r", bufs=1, name="rt")
                for c in range(IT):
                    nc.tensor.matmul(rt[:, c:c + 1],
                                     lhsT=r_sb[0:1, ts(c, 128)],
                                     rhs=ones1f[:],
                                     is_transpose=True,
                                     start=True, stop=True)
                nc.vector.reciprocal(rr_col[:], rt[:])
                nc.vector.tensor_copy(r_sbh[:], r_sb[:])

                # G2 half 0 -> gather 0
                for i in range(4):
                    nc.vector.scalar_tensor_tensor(
                        g2o[:, i * 512:(i + 1) * 512], ua0[i][:],
                        rr_col[:, i:i + 1], bks,
                        op0=AOP.mult, op1=AOP.add)
                nc.sync.dma_start(out=g_in0[:, :], in_=g2o[:, 0:4 * 512])
                nc.gpsimd.collective_compute(
                    "AllGather", AOP.bypass,
                    ins=[g_in0.ap().opt()],
                    outs=[g_all0.ap().opt()],
                    replica_groups=GROUPS,
                )

                # ---- hop A half 1 (i-tiles 4-7) -----------------------
                ua1 = [psA.tile([128, DOUT], f32, tag=f"ua{i}", bufs=1,
                                name=f"ua1_{i}") for i in range(4)]
                for jt in range(JT):
                    for i in range(4):
                        nc.tensor.matmul(ua1[i][:],
                                         lhsT=atS(jt, (4 + i) * 128, 128),
                                         rhs=p2S(jt),
                                         start=(jt == 0),
                                         stop=(jt == JT - 1))

                # r128 row-broadcast + xts (for the Wk0 term)
                r128 = psA.tile([128, HALF], f32, tag="r", bufs=1,
                                name="r128")
                for h in range(2):
                    nc.tensor.matmul(r128[:, ts(h, 512)], lhsT=ones128h[:],
                                     rhs=r_sbh[0:1, ts(h, 512)],
                                     start=True, stop=True)
                for d in range(DT):
                    nc.vector.tensor_mul(
                        xts_all[:, d * HALF:(d + 1) * HALF],
                        xt_all[:, d * N:d * N + HALF], r128[:])

                # G2 half 1 -> gather 1
                for i in range(4):
                    nc.vector.scalar_tensor_tensor(
                        g2o[:, (4 + i) * 512:(5 + i) * 512], ua1[i][:],
                        rr_col[:, 4 + i:5 + i], bks,
                        op0=AOP.mult, op1=AOP.add)
                nc.sync.dma_start(out=g_in1[:, :], in_=g2o[:, 4 * 512:8 * 512])
                nc.gpsimd.collective_compute(
                    "AllGather", AOP.bypass,
                    ins=[g_in1.ap().opt()],
                    outs=[g_all1.ap().opt()],
                    replica_groups=GROUPS,
                )

                # ---- P1: partner half first (copies), own half fused --
                for jt in list(range(IT, JT)) + list(range(IT)):
                    pp1 = psA.tile([128, DOUT], f32, tag="mm", bufs=2)
                    for d in range(DT):
                        nc.tensor.matmul(
                            pp1[:],
                            lhsT=xt_all[:, d * N + jt * 128:
                                        d * N + (jt + 1) * 128],
                            rhs=wk_all[:, (4 + d) * 512:(5 + d) * 512],
                            start=(d == 0), stop=(d == DT - 1))
                    if jt >= IT:
                        nc.scalar.copy(sS(jt), pp1[:])
                    else:
                        nc.vector.tensor_add(sS(jt), pp1[:],
                                             g2o[:, jt * 512:(jt + 1) * 512])

                # ---- S partner fix via masked adds --------------------
                nc.sync.dma_start(out=glo0[:], in_=g_all0[0:128, :])
                nc.sync.dma_start(out=ghi0[:], in_=g_all0[128:256, :])
                for t in range(4):
                    jt = IT + t
                    nc.vector.scalar_tensor_tensor(
                        sS(jt), glo0[:, t * 512:(t + 1) * 512], mlo, sS(jt),
                        op0=AOP.mult, op1=AOP.add)
                    nc.vector.scalar_tensor_tensor(
                        sS(jt), ghi0[:, t * 512:(t + 1) * 512], mhi, sS(jt),
                        op0=AOP.mult, op1=AOP.add)
                nc.sync.dma_start(out=glo1[:], in_=g_all1[0:128, :])
                nc.sync.dma_start(out=ghi1[:], in_=g_all1[128:256, :])
                for t in range(4):
                    jt = IT + 4 + t
                    nc.vector.scalar_tensor_tensor(
                        sS(jt), glo1[:, t * 512:(t + 1) * 512], mlo, sS(jt),
                        op0=AOP.mult, op1=AOP.add)
                    nc.vector.scalar_tensor_tensor(
                        sS(jt), ghi1[:, t * 512:(t + 1) * 512], mhi, sS(jt),
                        op0=AOP.mult, op1=AOP.add)

            # ---- phase 3: H = (r x X)@Wk0 + alphaT^T S ----------------
            with tc.tile_pool(name="psC", bufs=1, space="PSUM") as psC:
                hps = [psC.tile([128, DOUT], f32, tag=f"h{i}", bufs=1,
                                name=f"h{i}") for i in range(IT)]
                for it in range(IT):
                    for d in range(DT):
                        nc.tensor.matmul(
                            hps[it][:],
                            lhsT=xts_all[:, d * HALF + it * 128:
                                         d * HALF + (it + 1) * 128],
                            rhs=wk_all[:, (8 + d) * 512:(9 + d) * 512],
                            start=(d == 0), stop=False)
                # own-j chunk (S available pre-gather)
                for jt in range(IT):
                    for it in range(IT):
                        nc.tensor.matmul(hps[it][:],
                                         lhsT=atS(jt, it * 128, 128),
                                         rhs=sS(jt),
                                         start=False, stop=False)
                # gather0 chunk
                for jt in range(IT, IT + 4):
                    for it in range(IT):
                        nc.tensor.matmul(hps[it][:],
                                         lhsT=atS(jt, it * 128, 128),
                                         rhs=sS(jt),
                                         start=False, stop=False)
                # gather1 chunk, i-major with relu + OUT dribble
                for it in range(IT):
                    for jt in range(IT + 4, JT):
                        nc.tensor.matmul(hps[it][:],
                                         lhsT=atS(jt, it * 128, 128),
                                         rhs=sS(jt),
                                         start=False, stop=(jt == JT - 1))
                    nc.scalar.activation(o_all[:, it * 512:(it + 1) * 512],
                                         hps[it][:], AF.Relu,
                                         scale=rr_col[:, it:it + 1])
                    nc.sync.dma_start(out=OUT[ts(it, 128), :],
                                      in_=o_all[:, it * 512:(it + 1) * 512])

    nc.compile()
    return nc


def _prep_inputs(X, A, Wv, bv, aw, ab, Wk, bk):
    import ml_dtypes

    bf16 = ml_dtypes.bfloat16
    X = np.asarray(X, np.float32)
    A = np.asarray(A, np.float32)
    Wv = np.asarray(Wv, np.float32)
    bv = np.asarray(bv, np.float32)
    aw = np.asarray(aw, np.float32)
    ab = np.asarray(ab, np.float32)
    Wk = np.asarray(Wk, np.float32)
    bk = np.asarray(bk, np.float32)

    w1 = Wv @ aw[:DOUT, 0]
    c1 = float(bv @ aw[:DOUT, 0])
    w2 = Wv @ aw[DOUT:, 0]
    c2 = float(bv @ aw[DOUT:, 0]) + float(ab[0])
    bks = bk.sum(axis=0).astype(np.float32)

    def interleave(mat, tiles, cols):
        # [tiles*128, cols] -> [128, tiles*cols] with (p, t, c) order
        return np.ascontiguousarray(
            mat.reshape(tiles, 128, cols).transpose(1, 0, 2)
               .reshape(128, tiles * cols))

    # wk pack: wk2 then wk1 then wk0, each interleaved [128, 4*512]
    wkh = np.concatenate(
        [interleave(np.asarray(Wk[k], np.float32), DT, 512)
         for k in (2, 1, 0)], axis=1).astype(bf16)

    in_maps = []
    for c in range(NCORES):
        b, hf = c // 2, c % 2
        own = slice(hf * HALF, (hf + 1) * HALF)
        oth = slice((1 - hf) * HALF, (2 - hf) * HALF)
        perm = np.r_[np.arange(own.start, own.stop),
                     np.arange(oth.start, oth.stop)]
        Xb = X[b]
        sj = (Xb @ w1 + c1).astype(np.float32)
        si = (Xb @ w2 + c2).astype(np.float32)
        sjp = sj[perm]
        u1 = np.exp(sjp).astype(np.float32)
        u2 = np.exp(0.2 * sjp).astype(np.float32)
        sio = si[own]
        v1 = np.exp(sio).astype(np.float32)
        v2 = np.exp(0.2 * sio).astype(np.float32)

        smv = np.zeros((128, 546), np.float32)
        smv[:, 0:16] = u1.reshape(16, 128).T
        smv[:, 16:32] = u2.reshape(16, 128).T
        smv[:, 32:544] = bks[None, :]
        smv[:, 544] = 1.0 if hf == 1 else 0.0
        smv[:, 545] = 1.0 if hf == 0 else 0.0

        vvv = np.empty((128, 2 * HALF), np.float32)
        vvv[:, 0:HALF] = v1[None, :]
        vvv[:, HALF:] = v2[None, :]

        ath = interleave(np.ascontiguousarray(A[b][own, :].T[perm, :]),
                         JT, HALF).astype(bf16)
        xth = interleave(np.ascontiguousarray(Xb.T[:, perm]),
                         DT, N).astype(bf16)

        in_maps.append({
            "ATH": ath,
            "XTH": xth,
            "WKH": wkh,
            "SM": smv,
            "VV": vvv.astype(bf16),
        })
    return in_maps


LAST_RESULTS = None


def kernel(X, A, Wv, bv, aw, ab, Wk, bk):
    from concourse.bass_utils import run_bass_kernel_spmd

    if "nc" not in _CACHE:
        _CACHE["nc"] = _build()
    nc = _CACHE["nc"]

    in_maps = _prep_inputs(X, A, Wv, bv, aw, ab, Wk, bk)
    try:
        res = run_bass_kernel_spmd(nc, in_maps, core_ids=list(range(NCORES)))
    except Exception:
        import time
        time.sleep(20)
        res = run_bass_kernel_spmd(nc, in_maps, core_ids=list(range(NCORES)))
    global LAST_RESULTS
    LAST_RESULTS = res

    out = np.empty((B, N, DOUT), np.float32)
    for c in range(NCORES):
        b, hf = c // 2, c % 2
        out[b, hf * HALF:(hf + 1) * HALF, :] = res.results[c]["out"]
    return out


# revision 7
# speedup vs baseline: 1.0979x; 1.0212x over previous
"""Trainium2 Bass kernel for nn_ADCLayer (GAT-style message passing).

Math (reference reduction):
  sj = X @ (Wv @ aw[:d]) + bv.aw[:d]          (per-column score, j axis)
  si = X @ (Wv @ aw[d:]) + bv.aw[d:] + ab     (per-row score, i axis)
  alpha = A * exp(leaky_relu(si[i] + sj[j]))  (unnormalized transition)
  T = alpha / rowsum(alpha)
  H = X@Wk0 + (T X)@Wk1 + (T^2 X)@Wk2 + sum_k bk[k]   (last ref hop is dead code)
  out = relu(H)

Key identity used on device: exp is monotone, so
  exp(lrelu(x)) = max(exp(x), exp(0.2 x)),  and with x = si + sj both
  branches are rank-1:  exp(si+sj) = exp(si)*exp(sj).
The host precomputes u1=exp(sj), u2=exp(0.2 sj) (per-partition columns)
and V1=exp(si), V2=exp(0.2 si) (broadcast rows), so the device per j-tile
does just: m1 = u1*V1 (scalar engine), m2 = max(u2*V2, m1) (DVE stt),
alphaT = A*m2 (DVE) -- 3 cheap bf16 passes, no Exp LUT.

Device algebra (per core, partition=j layout, zero big transposes, both
hops run on RAW alphaT so nothing waits for normalization):
  alphaT[j, i] = A^T[j, i] * max(u1[j]V1[i], u2[j]V2[i])   (bf16)
  r via ones-stationary matmul -> (1, I); rr_col via 8 tiny PE
  transposes + exact reciprocal.
  P2 = X@Wk2 ; G2 = rr_col * (alphaT^T P2) + bks -> ONE pairwise
  AllGather (pair collectives cost ~30us latency; issue once, early,
  and hide behind P1 + P0 + hopB own-j work).
  S = P1 + G2 (own fused from PSUM; partner via masked add).
  H_psum = (r*X)@Wk0 + alphaT^T S ; out = relu(rr_col * H_psum), bf16.

Sharding: 8 cores = 4 batches x 2 row-halves; j axis permuted per core
(own half first) so own j-tiles have uniform local indices.

Schedule notes:
- few big DMAs on 2 HW queues, priority-ordered so the first r/P2
  matmuls start ~10us in (X is shipped jt-major for P1/P2 so the first
  jt chunk lands early; a second d-major own-half copy feeds xts/Wk0).
- PE emission: per jt [r, P2] interleaved with the elementwise pipe,
  p2 psum->sbuf copies run on the scalar queue 2 tiles behind.
- hopA in two 4-bank PSUM halves; G2 for both halves -> one gather.
- P1 emitted partner-half first (plain copies) then own-half (fused
  S-own adds) so nothing blocks on the gather.
- S partner fixes happen OUTSIDE the psA pool scope so the psC pool
  (phase 3) opens as soon as P1's psum is drained -- phase 3 must not
  wait on the collective.
- hopB j-chunked: P0, own j 0-7, partner j 8-11, then j 12-15 i-major
  with per-i relu + OUT DMA dribble.
"""

import numpy as np

B, N, DIN, DOUT = 4, 2048, 512, 512
HALF = N // 2          # rows per core
NCORES = 8
JT = N // 128          # 16 j tiles
IT = HALF // 128       # 8 i tiles (also own j tiles)
DT = DIN // 128        # 4 d tiles

_CACHE = {}


def _build():
    import concourse.bacc as bacc
    import concourse.tile as tile
    import concourse.mybir as mybir
    from concourse.bass import ds, ts

    f32 = mybir.dt.float32
    bf16 = mybir.dt.bfloat16
    AOP = mybir.AluOpType
    AF = mybir.ActivationFunctionType

    nc = bacc.Bacc("TRN2", target_bir_lowering=False, debug=False,
                   num_devices=NCORES)

    ATH = nc.declare_dram_parameter("ATH", [128, JT * HALF], bf16,
                                    isOutput=False)
    # X^T, jt-major interleave: [p, jt, d, 128] (for P1/P2 lhsT slices)
    XTJ = nc.declare_dram_parameter("XTJ", [128, JT * DIN], bf16,
                                    isOutput=False)
    # X^T own half, d-major: [p, d, i] (for xts / Wk0 term)
    XTO = nc.declare_dram_parameter("XTO", [128, DT * HALF], bf16,
                                    isOutput=False)
    # wk order: wk2 d0..3, wk1 d0..3, wk0 d0..3
    WKH = nc.declare_dram_parameter("WKH", [128, 12 * 512], bf16,
                                    isOutput=False)
    # smalls: u1(16) u2(16) bks(512) mlo(1) mhi(1)
    SM = nc.declare_dram_parameter("SM", [128, 546], f32, isOutput=False)
    # V1(1024) V2(1024) broadcast rows
    VV = nc.declare_dram_parameter("VV", [128, 2 * HALF], bf16,
                                   isOutput=False)
    OUT = nc.declare_dram_parameter("out", [HALF, DOUT], bf16, isOutput=True)

    g_in = nc.dram_tensor("g_in", [128, IT * 512], bf16)
    g_all = nc.dram_tensor("g_all", [256, IT * 512], bf16)

    GROUPS = [[0, 1], [2, 3], [4, 5], [6, 7]]

    with tile.TileContext(nc) as tc:
        with tc.tile_pool(name="sb", bufs=1) as sb:
            # ---- big SBUF tiles ---------------------------------------
            at_all = sb.tile([128, JT * HALF], bf16, tag="at", bufs=1)
            xtj = sb.tile([128, JT * DIN], bf16, tag="xtj", bufs=1)
            xto = sb.tile([128, DT * HALF], bf16, tag="xto", bufs=1)
            wk_all = sb.tile([128, 12 * 512], bf16, tag="wk", bufs=1)
            sm = sb.tile([128, 546], f32, tag="sm", bufs=1)
            vv = sb.tile([128, 2 * HALF], bf16, tag="vv", bufs=1)
            p2_all = sb.tile([128, JT * 512], bf16, tag="p2", bufs=1)
            s_all = sb.tile([128, JT * 512], bf16, tag="s", bufs=1)
            g2o = sb.tile([128, IT * 512], bf16, tag="g2o", bufs=1)
            glo = sb.tile([128, IT * 512], bf16, tag="glo", bufs=1)
            ghi = sb.tile([128, IT * 512], bf16, tag="ghi", bufs=1)
            xts_all = sb.tile([128, DT * HALF], bf16, tag="xts", bufs=1)
            o_all = sb.tile([128, IT * 512], bf16, tag="o", bufs=1)
            r_sb = sb.tile([1, HALF], f32, tag="rsb", bufs=1)
            r_sbh = sb.tile([1, HALF], bf16, tag="rsbh", bufs=1)
            rr_col = sb.tile([128, IT], f32, tag="rrc", bufs=1)

            # ---- input DMAs: priority-ordered, few big issues ---------
            # sync queue feeds the elementwise pipe (SM, V1, V2, A chunks)
            nc.sync.dma_start(out=sm[:], in_=SM[:, :])
            nc.sync.dma_start(out=vv[:, 0:HALF], in_=VV[:, 0:HALF])
            nc.sync.dma_start(out=vv[:, HALF:2 * HALF],
                              in_=VV[:, HALF:2 * HALF])
            nc.sync.dma_start(out=at_all[:, 0:HALF], in_=ATH[:, 0:HALF])
            nc.sync.dma_start(out=at_all[:, HALF:4 * HALF],
                              in_=ATH[:, HALF:4 * HALF])
            nc.sync.dma_start(out=at_all[:, 4 * HALF:10 * HALF],
                              in_=ATH[:, 4 * HALF:10 * HALF])
            nc.sync.dma_start(out=at_all[:, 10 * HALF:JT * HALF],
                              in_=ATH[:, 10 * HALF:JT * HALF])
            # scalar queue feeds the PE (wk2, X jt-chunks, rest)
            nc.scalar.dma_start(out=wk_all[:, 0:4 * 512],
                                in_=WKH[:, 0:4 * 512])
            nc.scalar.dma_start(out=xtj[:, 0:4 * DIN], in_=XTJ[:, 0:4 * DIN])
            nc.scalar.dma_start(out=xtj[:, 4 * DIN:JT * DIN],
                                in_=XTJ[:, 4 * DIN:JT * DIN])
            nc.scalar.dma_start(out=xto[:], in_=XTO[:, :])
            nc.scalar.dma_start(out=wk_all[:, 4 * 512:12 * 512],
                                in_=WKH[:, 4 * 512:12 * 512])

            ones = sb.tile([128, 1], bf16, tag="ones", bufs=1)
            nc.vector.memset(ones[:], 1.0)
            ones1f = sb.tile([1, 1], f32, tag="ones1f", bufs=1)
            nc.vector.memset(ones1f[:], 1.0)
            ones128h = sb.tile([1, 128], bf16, tag="o128", bufs=1)
            nc.vector.memset(ones128h[:], 1.0)

            def atS(jt, off, size):
                return at_all[:, jt * HALF + off: jt * HALF + off + size]

            def xjS(jt, d):
                return xtj[:, jt * DIN + d * 128: jt * DIN + (d + 1) * 128]

            def p2S(jt):
                return p2_all[:, jt * 512:(jt + 1) * 512]

            def sS(jt):
                return s_all[:, jt * 512:(jt + 1) * 512]

            V1 = vv[:, 0:HALF]
            V2 = vv[:, HALF:2 * HALF]
            bks = sm[:, 32:544]
            mlo = sm[:, 544:545]
            mhi = sm[:, 545:546]

            with tc.tile_pool(name="psA", bufs=1, space="PSUM") as psA:
                # ---- phase 1: elementwise alphaT + r + P2 -------------
                r_ps = psA.tile([1, HALF], f32, tag="r", bufs=1)
                pp2_tiles = []
                for jt in range(JT):
                    m1 = sb.tile([128, HALF], bf16, tag="m1", bufs=3)
                    nc.scalar.activation(m1[:], V1, AF.Identity,
                                         scale=sm[:, jt:jt + 1])
                    m2 = sb.tile([128, HALF], bf16, tag="m2", bufs=3)
                    nc.vector.scalar_tensor_tensor(
                        m2[:], V2, sm[:, 16 + jt:17 + jt], m1[:],
                        op0=AOP.mult, op1=AOP.max)
                    nc.vector.tensor_mul(atS(jt, 0, HALF), atS(jt, 0, HALF),
                                         m2[:])
                    # r rowsum (PE, interleaved with P2)
                    for h in range(2):
                        nc.tensor.matmul(r_ps[:, ts(h, 512)], lhsT=ones[:],
                                         rhs=atS(jt, h * 512, 512),
                                         start=(jt == 0), stop=(jt == JT - 1))
                    # P2 tile jt (psum->sbuf copy deferred 2 tiles so the
                    # scalar queue never head-blocks the elementwise pipe)
                    if jt >= 2:
                        nc.scalar.copy(p2S(jt - 2), pp2_tiles[jt - 2][:])
                    pp2 = psA.tile([128, DOUT], f32, tag="mm", bufs=2,
                                   name=f"pp2_{jt}")
                    for d in range(DT):
                        nc.tensor.matmul(
                            pp2[:], lhsT=xjS(jt, d),
                            rhs=wk_all[:, d * 512:(d + 1) * 512],
                            start=(d == 0), stop=(d == DT - 1))
                    pp2_tiles.append(pp2)
                nc.scalar.copy(p2S(JT - 2), pp2_tiles[JT - 2][:])
                nc.scalar.copy(p2S(JT - 1), pp2_tiles[JT - 1][:])

                # ---- hop A half 0 (i-tiles 0-3) -----------------------
                ua0 = [psA.tile([128, DOUT], f32, tag=f"ua{i}", bufs=1,
                                name=f"ua0_{i}") for i in range(4)]
                for jt in range(JT):
                    for i in range(4):
                        nc.tensor.matmul(ua0[i][:],
                                         lhsT=atS(jt, i * 128, 128),
                                         rhs=p2S(jt),
                                         start=(jt == 0),
                                         stop=(jt == JT - 1))

                # r -> rr_col (8 tiny PE transposes + exact reciprocal)
                nc.vector.tensor_copy(r_sb[:], r_ps[:])
                rt = psA.tile([128, IT], f32, tag="r", bufs=1, name="rt")
                for c in range(IT):
                    nc.tensor.matmul(rt[:, c:c + 1],
                                     lhsT=r_sb[0:1, ts(c, 128)],
                                     rhs=ones1f[:],
                                     is_transpose=True,
                                     start=True, stop=True)
                nc.vector.reciprocal(rr_col[:], rt[:])
                nc.vector.tensor_copy(r_sbh[:], r_sb[:])

                # G2 half 0
                for i in range(4):
                    nc.vector.scalar_tensor_tensor(
                        g2o[:, i * 512:(i + 1) * 512], ua0[i][:],
                        rr_col[:, i:i + 1], bks,
                        op0=AOP.mult, op1=AOP.add)

                # ---- hop A half 1 (i-tiles 4-7) -----------------------
                ua1 = [psA.tile([128, DOUT], f32, tag=f"ua{i}", bufs=1,
                                name=f"ua1_{i}") for i in range(4)]
                for jt in range(JT):
                    for i in range(4):
                        nc.tensor.matmul(ua1[i][:],
                                         lhsT=atS(jt, (4 + i) * 128, 128),
                                         rhs=p2S(jt),
                                         start=(jt == 0),
                                         stop=(jt == JT - 1))

                # r128 row-broadcast + xts (for the Wk0 term)
                r128 = psA.tile([128, HALF], f32, tag="r", bufs=1,
                                name="r128")
                for h in range(2):
                    nc.tensor.matmul(r128[:, ts(h, 512)], lhsT=ones128h[:],
                                     rhs=r_sbh[0:1, ts(h, 512)],
                                     start=True, stop=True)
                for d in range(DT):
                    nc.vector.tensor_mul(
                        xts_all[:, d * HALF:(d + 1) * HALF],
                        xto[:, d * HALF:(d + 1) * HALF], r128[:])

                # G2 half 1 -> single gather for all 8 tiles
                for i in range(4):
                    nc.vector.scalar_tensor_tensor(
                        g2o[:, (4 + i) * 512:(5 + i) * 512], ua1[i][:],
                        rr_col[:, 4 + i:5 + i], bks,
                        op0=AOP.mult, op1=AOP.add)
                nc.sync.dma_start(out=g_in[:, :], in_=g2o[:, :])
                nc.gpsimd.collective_compute(
                    "AllGather", AOP.bypass,
                    ins=[g_in.ap().opt()],
                    outs=[g_all.ap().opt()],
                    replica_groups=GROUPS,
                )

                # ---- P1: partner half first (copies), own half fused --
                for jt in list(range(IT, JT)) + list(range(IT)):
                    pp1 = psA.tile([128, DOUT], f32, tag="mm", bufs=2,
                                   name=f"pp1_{jt}")
                    for d in range(DT):
                        nc.tensor.matmul(
                            pp1[:], lhsT=xjS(jt, d),
                            rhs=wk_all[:, (4 + d) * 512:(5 + d) * 512],
                            start=(d == 0), stop=(d == DT - 1))
                    if jt >= IT:
                        nc.scalar.copy(sS(jt), pp1[:])
                    else:
                        nc.vector.tensor_add(sS(jt), pp1[:],
                                             g2o[:, jt * 512:(jt + 1) * 512])

            # ---- S partner fix (outside psA so phase 3 need not wait) -
            nc.sync.dma_start(out=glo[:], in_=g_all[0:128, :])
            nc.sync.dma_start(out=ghi[:], in_=g_all[128:256, :])
            for t in range(IT):
                jt = IT + t
                nc.vector.scalar_tensor_tensor(
                    sS(jt), glo[:, t * 512:(t + 1) * 512], mlo, sS(jt),
                    op0=AOP.mult, op1=AOP.add)
                nc.vector.scalar_tensor_tensor(
                    sS(jt), ghi[:, t * 512:(t + 1) * 512], mhi, sS(jt),
                    op0=AOP.mult, op1=AOP.add)

            # ---- phase 3: H = (r x X)@Wk0 + alphaT^T S ----------------
            with tc.tile_pool(name="psC", bufs=1, space="PSUM") as psC:
                hps = [psC.tile([128, DOUT], f32, tag=f"h{i}", bufs=1,
                                name=f"h{i}") for i in range(IT)]
                for it in range(IT):
                    for d in range(DT):
                        nc.tensor.matmul(
                            hps[it][:],
                            lhsT=xts_all[:, d * HALF + it * 128:
                                         d * HALF + (it + 1) * 128],
                            rhs=wk_all[:, (8 + d) * 512:(9 + d) * 512],
                            start=(d == 0), stop=False)
                # own-j chunk (S available pre-gather)
                for jt in range(IT):
                    for it in range(IT):
                        nc.tensor.matmul(hps[it][:],
                                         lhsT=atS(jt, it * 128, 128),
                                         rhs=sS(jt),
                                         start=False, stop=False)
                # partner chunk part 1
                for jt in range(IT, IT + 4):
                    for it in range(IT):
                        nc.tensor.matmul(hps[it][:],
                                         lhsT=atS(jt, it * 128, 128),
                                         rhs=sS(jt),
                                         start=False, stop=False)
                # partner tail, i-major with relu + OUT dribble
                for it in range(IT):
                    for jt in range(IT + 4, JT):
                        nc.tensor.matmul(hps[it][:],
                                         lhsT=atS(jt, it * 128, 128),
                                         rhs=sS(jt),
                                         start=False, stop=(jt == JT - 1))
                    nc.scalar.activation(o_all[:, it * 512:(it + 1) * 512],
                                         hps[it][:], AF.Relu,
                                         scale=rr_col[:, it:it + 1])
                    nc.sync.dma_start(out=OUT[ts(it, 128), :],
                                      in_=o_all[:, it * 512:(it + 1) * 512])

    nc.compile()
    return nc


def _prep_inputs(X, A, Wv, bv, aw, ab, Wk, bk):
    import ml_dtypes

    bf16 = ml_dtypes.bfloat16
    X = np.asarray(X, np.float32)
    A = np.asarray(A, np.float32)
    Wv = np.asarray(Wv, np.float32)
    bv = np.asarray(bv, np.float32)
    aw = np.asarray(aw, np.float32)
    ab = np.asarray(ab, np.float32)
    Wk = np.asarray(Wk, np.float32)
    bk = np.asarray(bk, np.float32)

    w1 = Wv @ aw[:DOUT, 0]
    c1 = float(bv @ aw[:DOUT, 0])
    w2 = Wv @ aw[DOUT:, 0]
    c2 = float(bv @ aw[DOUT:, 0]) + float(ab[0])
    bks = bk.sum(axis=0).astype(np.float32)

    def interleave(mat, tiles, cols):
        # [tiles*128, cols] -> [128, tiles*cols] with (p, t, c) order
        return np.ascontiguousarray(
            mat.reshape(tiles, 128, cols).transpose(1, 0, 2)
               .reshape(128, tiles * cols))

    # wk pack: wk2 then wk1 then wk0, each interleaved [128, 4*512]
    wkh = np.concatenate(
        [interleave(np.asarray(Wk[k], np.float32), DT, 512)
         for k in (2, 1, 0)], axis=1).astype(bf16)

    in_maps = []
    for c in range(NCORES):
        b, hf = c // 2, c % 2
        own = slice(hf * HALF, (hf + 1) * HALF)
        oth = slice((1 - hf) * HALF, (2 - hf) * HALF)
        perm = np.r_[np.arange(own.start, own.stop),
                     np.arange(oth.start, oth.stop)]
        Xb = X[b]
        sj = (Xb @ w1 + c1).astype(np.float32)
        si = (Xb @ w2 + c2).astype(np.float32)
        sjp = sj[perm]
        u1 = np.exp(sjp).astype(np.float32)
        u2 = np.exp(0.2 * sjp).astype(np.float32)
        sio = si[own]
        v1 = np.exp(sio).astype(np.float32)
        v2 = np.exp(0.2 * sio).astype(np.float32)

        smv = np.zeros((128, 546), np.float32)
        smv[:, 0:16] = u1.reshape(16, 128).T
        smv[:, 16:32] = u2.reshape(16, 128).T
        smv[:, 32:544] = bks[None, :]
        smv[:, 544] = 1.0 if hf == 1 else 0.0
        smv[:, 545] = 1.0 if hf == 0 else 0.0

        vvv = np.empty((128, 2 * HALF), np.float32)
        vvv[:, 0:HALF] = v1[None, :]
        vvv[:, HALF:] = v2[None, :]

        ath = interleave(np.ascontiguousarray(A[b][own, :].T[perm, :]),
                         JT, HALF).astype(bf16)
        XTp = np.ascontiguousarray(Xb.T[:, perm])        # [512, 2048]
        # jt-major: [p, jt, d, 128]
        xtj = np.ascontiguousarray(
            XTp.reshape(DT, 128, JT, 128).transpose(1, 2, 0, 3)
               .reshape(128, JT * DIN)).astype(bf16)
        # d-major own half: [p, d, i]
        xto = interleave(XTp[:, 0:HALF], DT, HALF).astype(bf16)

        in_maps.append({
            "ATH": ath,
            "XTJ": xtj,
            "XTO": xto,
            "WKH": wkh,
            "SM": smv,
            "VV": vvv.astype(bf16),
        })
    return in_maps


LAST_RESULTS = None


def kernel(X, A, Wv, bv, aw, ab, Wk, bk):
    from concourse.bass_utils import run_bass_kernel_spmd

    if "nc" not in _CACHE:
        _CACHE["nc"] = _build()
    nc = _CACHE["nc"]

    in_maps = _prep_inputs(X, A, Wv, bv, aw, ab, Wk, bk)
    try:
        res = run_bass_kernel_spmd(nc, in_maps, core_ids=list(range(NCORES)))
    except Exception:
        import time
        time.sleep(20)
        res = run_bass_kernel_spmd(nc, in_maps, core_ids=list(range(NCORES)))
    global LAST_RESULTS
    LAST_RESULTS = res

    out = np.empty((B, N, DOUT), np.float32)
    for c in range(NCORES):
        b, hf = c // 2, c % 2
        out[b, hf * HALF:(hf + 1) * HALF, :] = res.results[c]["out"]
    return out
